# revision 110
# baseline (speedup 1.0000x reference)
"""Trainium2 Bass kernel for a post-norm transformer encoder layer with RoPE.

B=2, S=2048, D=1024, H=16, Dh=64, F=4096, fp32 in/out.

Sharding (8 cores, no collectives): core c handles batch b=c//4 and query block
qb=c%4 (512 queries). Each core recomputes K/V for its full batch, computes
Q/attention/out_proj/LN/FFN for its own 512 rows.

v4 design (vs v2 baseline, 556us -> ~446us HW):
  - Rope tables / bias pack / ow loaded ONCE into a persistent SBUF pool;
    x chunk 0 and wq live in persistent single-slot pools whose NEXT-rep
    reload DMAs are emitted mid-rep (as soon as this rep's readers drain),
    so the rep boundary has no DMA wait. wk/wv/x1-3 stream per rep; w1/w2
    stream through double-buffered blocks inside phase F.
  - Phases A (K^T proj + rope) and B (V proj) fused into one s-chunk-major
    loop: each x chunk is consumed once, so x1-3 stream through 2 rotating
    8K slots instead of 24K resident.
  - RoPE 32-row swap via a PE permutation matmul (P_sign, rope signs folded
    into the matrix) instead of gpsimd SBUF->SBUF DMAs: the single SWDGE
    queue serialized phase C at ~1us/descriptor. sin table holds |sin|.
  - No f32 x copy: the attention residual reads the bf16 x chunk via a
    fused scalar_tensor_tensor (psum + bias + residual in one DVE op).
  - LN chains compressed: sum-stats matmuls emitted right after out_proj
    (not interleaved, so PE never waits per-ot); var = stt(pSq,1/D,-mu^2)
    in one op; +eps folded into Ln's bias; LN1's +be1 folded into b1
    (b1 + w1 @ be1) and the FFN residual bias (b2 + be1) on the host, so
    normalize is just sub + stt -> bf16 H1b (subs alternate DVE/Pool).
  - Pool open ORDER chosen so every pool's region-WAR falls on tiles whose
    readers finish before its first write (SBUF is oversubscribed, so the
    left F-pools physically overlap the next rep's right-stack pools):
    w2p on HR's slot, w1p on space free since the prior rep, scrF (LN2 yt,
    read by the stores) deepest; LN1 scratch on the RIGHT stack so w1b's
    prefetch never WAR-waits on the LN1 tail.
  - Output stores ride the SWDGE (gpsimd) queue: on either HWDGE ring
    their store-gated entries would head-of-line block next-rep loads.
  - Attention: ACT does ONLY the exp (128x [128,1024] psum->bf16); softmax
    denom via the V-augmented ones row; reciprocal + crossed-base normalize
    on DVE; out_proj interleaved so PE fills ACT-bound gaps.
"""
import sys, os
for _p in ('/opt/trn_rl_repo', '/root/.axon_site/_ro/trn_rl_repo'):
    if os.path.isdir(_p) and _p not in sys.path:
        sys.path.insert(0, _p)

import numpy as np
import ml_dtypes
from contextlib import ExitStack

import concourse.bacc as bacc
import concourse.mybir as mybir
import concourse.tile as tile
from concourse.bass_utils import run_bass_kernel_spmd

# Pin every activation this kernel uses to the single table set that covers
# them all (natural_log_exp_and_others: exp+ln+identity+relu+square), by
# stripping those functions from every other set before the table-load pass
# runs. Otherwise the chooser alternates exp_and_others <-> natural_log and
# inserts ~4 LoadActFuncSet (~2.7us each) per rep around the LayerNorms.
import functools as _ft
import concourse.hw_specs as _hw

_PIN_SET = "natural_log_exp_and_others"


@_ft.cache
def _pinned_act_tables(arch):
    tabs = {k: set(v) for k, v in _hw.get_activation_tables(arch).items()}
    if _PIN_SET in tabs:
        _AF = mybir.ActivationFunctionType
        ours = {_AF.Exp, _AF.Ln, _AF.Identity, _AF.Relu, _AF.Square}
        if ours <= tabs[_PIN_SET]:
            for name, funcs in tabs.items():
                if name != _PIN_SET:
                    funcs -= ours
    return tabs


bacc.get_activation_tables = _pinned_act_tables

F32 = mybir.dt.float32
F32R = mybir.dt.float32r
BF16 = mybir.dt.bfloat16
AF = mybir.ActivationFunctionType
ALU = mybir.AluOpType

B, S, D, H, Dh, F = 2, 2048, 1024, 16, 64, 4096
Q = 512                 # queries per core
NT_D = D // 128         # 8 d-tiles
NT_S = S // 128         # 16 s-tiles
NC_S = S // 512         # 4 s-chunks
NT_F = F // 128         # 32 f-tiles
LN_EPS = 1e-5
ROPE_BASE = 10000.0

# bpack column layout (each vector of length 1024 -> 8 cols, one per d-tile)
_BP = {"bq": 0, "bk": 8, "ob": 16, "b2": 24, "g1": 32, "be1": 40,
       "g2": 48, "be2": 56, "b1": 64, "eps": 88, "ones": 96}
BP_COLS = 112

_CACHE = {}


def _build(repeat=1):
    nc = bacc.Bacc("TRN2", target_bir_lowering=False, debug=False, num_devices=8)

    def inp(name, shape, dt):
        return nc.dram_tensor(name, list(shape), dt, kind="ExternalInput")

    # packed x (bf16): [p, sc*4096 + kt*512 + j]; chunk 0 is this core's
    # q-block (host permutes chunks; K rope tables are permuted to match)
    xPb = inp("xPb", (128, NC_S * 4096), BF16)
    wqP = inp("wqP", (128, D * NT_D), BF16)  # [p, kt*1024 + c] = wT[kt*128+p, c]
    wkP = inp("wkP", (128, D * NT_D), BF16)
    wvP = inp("wvP", (128, D * NT_D), BF16)
    owP = inp("owP", (128, D * NT_D), BF16)
    w1P = inp("w1P", (128, NT_D * F), BF16)  # w1T packed: [p, kt*F + c]
    w2P = inp("w2P", (128, NT_F * D), BF16)  # w2T packed: [p, ft*D + c]
    bpack = inp("bpack", (128, BP_COLS), F32R)
    cosKb = inp("cosKb", (128, S), BF16)     # [cos;cos;cos;cos] blocks of 32
    sinKAb = inp("sinKAb", (128, S), BF16)   # [sin;sin;sin;sin] (abs)
    psignT = inp("psignT", (128, 128), BF16)  # signed 32-row swap permutation
    identT = inp("identT", (128, 128), BF16)  # identity (psum residual adds)
    yT = nc.dram_tensor("yT", [D, Q], F32, kind="ExternalOutput")

    with tile.TileContext(nc) as tc, ExitStack() as octx:
        pconst = octx.enter_context(tc.tile_pool(name="pconst", bufs=1))

        bp = pconst.tile([128, BP_COLS], F32R, tag="bp")
        nc.sync.dma_start(bp[:], bpack.ap())
        psign_t = pconst.tile([128, 128], BF16, tag="psign")
        nc.sync.dma_start(psign_t[:], psignT.ap())
        ident_t = pconst.tile([128, 128], BF16, tag="ident")
        nc.sync.dma_start(ident_t[:], identT.ap())
        ow_t = pconst.tile([128, 8192], BF16, tag="ow")
        nc.sync.dma_start(ow_t[:], owP.ap())
        cosk_t = pconst.tile([128, S], BF16, tag="cosk")
        nc.scalar.dma_start(cosk_t[:], cosKb.ap())
        sink_t = pconst.tile([128, S], BF16, tag="sink")
        nc.scalar.dma_start(sink_t[:], sinKAb.ap())


        def bcol(key, i):          # (128,1) f32 bias view
            c = _BP[key] + i
            return bp[:, c:c + 1].bitcast(F32)

        ones_c = bp[:, _BP["ones"]:_BP["ones"] + 1]   # f32r stationary
        ones16 = bp[:, _BP["ones"]:_BP["ones"] + 16]  # f32r ones cols
        eps_c = bp[0:1, _BP["eps"]:_BP["eps"] + 1].bitcast(F32)
        # block 0 of the permuted tables IS this core's q-block
        cosq_t, sinq_t = cosk_t[:, 0:Q], sink_t[:, 0:Q]

        # Persistent single-slot pools for the two tensors the NEXT rep needs
        # first (x chunk 0 and wq): their reload DMAs are emitted mid-rep, as
        # soon as the current rep's readers drain, so the next rep's Q
        # projection never waits on DMA.
        px0 = octx.enter_context(tc.tile_pool(name="px0", bufs=1,
                                              side="right"))
        # 2 rotating x0 slots: the next-rep prefetch lands in the other slot,
        # so it never WAR-waits on this rep's attention-residual reads.
        pwq = octx.enter_context(tc.tile_pool(name="pwq", bufs=1,
                                              side="right"))
        pQT = octx.enter_context(tc.tile_pool(name="pQT", bufs=1,
                                              side="right"))

        def load_x0():
            t = px0.tile([128, 4096], BF16, tag="x0", name="xc0")
            nc.scalar.dma_start(t[:], xPb.ap()[:, 0:4096])
            return t

        def load_wq():
            t = pwq.tile([128, 8192], BF16, tag="wq", name="wqt")
            nc.sync.dma_start(t[:], wqP.ap())
            return t

        # Q^T projection + rope, software-pipelined: rep r+1's copy runs
        # inside rep r's LN1 bubble. Split in two parts so the matmul half
        # (PE + ACT) fills the bubble while LN1's DVE normalize runs, and
        # the rope half (DVE-heavy) defers into phase F's DVE slack.
        def phase_C_mm(x0, wq, rt):
            # QT tiles double as the pre-rope scratch: ACT writes wq@x + bq
            # into them, and phase_C_rope rotates them in place.
            QT = [pQT.tile([128, Q], BF16, tag=f"qt{i}", name=f"QT{rt}_{i}")
                  for i in range(NT_D)]
            with tc.tile_pool(name="psC", bufs=1, space="PSUM") as psC:
                for dt in range(NT_D):
                    pq = psC.tile([128, Q], F32, tag="pq", bufs=2,
                                  name=f"pqC{rt}_{dt}")
                    for kt in range(NT_D):
                        nc.tensor.matmul(
                            pq[:],
                            lhsT=wq[:, kt * 1024 + dt * 128:
                                    kt * 1024 + (dt + 1) * 128],
                            rhs=x0[:, kt * 512:(kt + 1) * 512],
                            start=(kt == 0), stop=(kt == NT_D - 1))
                    nc.scalar.activation(QT[dt][:], pq[:], AF.Identity,
                                         bias=bcol("bq", dt))
            return QT

        def phase_C_rope(QT, scr, rt):
            with tc.tile_pool(name="psWC", bufs=1, space="PSUM") as psW:
                for dt in range(NT_D):
                    psw = psW.tile([128, Q], F32, tag="psw", bufs=2,
                                   name=f"pswC{rt}_{dt}")
                    nc.tensor.matmul(psw[:], lhsT=psign_t[:],
                                     rhs=QT[dt][:], start=True, stop=True)
                    sw = scr.tile([128, Q], BF16, tag="swq", bufs=2,
                                  name=f"swC{rt}_{dt}")
                    nc.vector.tensor_mul(sw[:], psw[:], sinq_t)
                    nc.vector.tensor_mul(QT[dt][:], QT[dt][:], cosq_t)
                    nc.vector.tensor_add(QT[dt][:], QT[dt][:], sw[:])

        x0_t = load_x0()
        wq_t = load_wq()

        for _rep in range(repeat):
            # Right-stack pools, opened in reverse-close order (LIFO):
            #   s_EF (H1b)       closes at rep end    -> leftmost
            #   s_HR (HR)        closes after LN1
            #   s_big (KT/VA/..) closes after D+LN1
            #   s_x  (x stream + wk/wv) closes after the fused K/V loop
            s_EF = ExitStack()
            s_HR = ExitStack()
            s_big = ExitStack()
            s_x = ExitStack()

            h1p = s_EF.enter_context(tc.tile_pool(name="pH1b", bufs=1))
            hrp = s_HR.enter_context(tc.tile_pool(name="pHR", bufs=1))
            pKT = s_big.enter_context(tc.tile_pool(name="pKT", bufs=1,
                                                   side="right"))
            pVA = s_big.enter_context(tc.tile_pool(name="pVA", bufs=1,
                                                   side="right"))
            pATT = s_big.enter_context(tc.tile_pool(name="pATT", bufs=1,
                                                    side="right"))
            pxr = s_x.enter_context(tc.tile_pool(name="pxr", bufs=1,
                                                 side="right"))
            pwkv = s_x.enter_context(tc.tile_pool(name="pwkv", bufs=1,
                                                  side="right"))

            # ---- prologue DMAs: x chunks 1-3 stream through 2 rotating
            # slots on the ACT HWDGE ring; wk/wv on the SP ring ----
            wk_t = pwkv.tile([128, 8192], BF16, tag="wk", name="wkt")
            nc.sync.dma_start(wk_t[:], wkP.ap())
            wv_t = pwkv.tile([128, 8192], BF16, tag="wv", name="wvt")
            nc.sync.dma_start(wv_t[:], wvP.ap())
            x_t = [x0_t]
            for sc in range(1, NC_S):
                t = pxr.tile([128, 4096], BF16, tag="xr", bufs=2,
                             name=f"xc{sc}")
                nc.scalar.dma_start(t[:], xPb.ap()[:, sc * 4096:(sc + 1) * 4096])
                x_t.append(t)

            KT = [pKT.tile([128, S], BF16, tag=f"kt{i}", name=f"KT{i}")
                  for i in range(NT_D)]
            VA = [pVA.tile([128, 16 * 65], BF16, tag=f"va{i}", name=f"VA{i}")
                  for i in range(NT_S)]

            # ============ Phase C: Q^T proj + rope (qb chunk) ============
            with tc.tile_pool(name="scrC", bufs=1, side="right") as scrC:
                QT = phase_C_mm(x0_t, wq_t, f"c{_rep}")
                phase_C_rope(QT, scrC, f"c{_rep}")
            # wq is dead after phase C: prefetch next rep's copy now (SP
            # ring, ahead of w1; transfers while this rep computes).
            if _rep + 1 < repeat:
                wq_next = load_wq()

            # ===== Fused phase A+B: per s-chunk, K^T proj + rope, then V
            # proj — so each x chunk is consumed once and its slot can
            # stream the next chunk in. =====
            with ExitStack() as ctx:
                scr = ctx.enter_context(tc.tile_pool(name="scrA", bufs=1))
                psA = ctx.enter_context(tc.tile_pool(name="psA", bufs=1,
                                                     space="PSUM"))
                psW = ctx.enter_context(tc.tile_pool(name="psWA", bufs=1,
                                                     space="PSUM"))
                psB = ctx.enter_context(tc.tile_pool(name="psB", bufs=1,
                                                     space="PSUM"))

                def rope_k(dt, sc, raw):
                    c0 = sc * 512
                    psw = psW.tile([128, 512], F32, tag="pswa", bufs=2,
                                   name=f"pswA{dt}_{sc}")
                    nc.tensor.matmul(psw[:], lhsT=psign_t[:], rhs=raw[:],
                                     start=True, stop=True)
                    sw = scr.tile([128, 512], BF16, tag="swk", bufs=2,
                                  name=f"swA{dt}_{sc}")
                    nc.vector.tensor_mul(sw[:], psw[:], sink_t[:, c0:c0 + 512])
                    nc.vector.tensor_mul(raw[:], raw[:], cosk_t[:, c0:c0 + 512])
                    nc.vector.tensor_add(KT[dt][:, c0:c0 + 512], raw[:], sw[:])

                for sc in range(NC_S):
                    xs = x_t[sc]
                    pend = None   # rope staggered one dt behind the matmuls
                    for dt in range(NT_D):
                        pk = psA.tile([128, 512], F32, tag="pk", bufs=2,
                                      name=f"pkA{dt}_{sc}")
                        for kt in range(NT_D):
                            nc.tensor.matmul(
                                pk[:],
                                lhsT=wk_t[:, kt * 1024 + dt * 128:
                                          kt * 1024 + (dt + 1) * 128],
                                rhs=xs[:, kt * 512:(kt + 1) * 512],
                                start=(kt == 0), stop=(kt == NT_D - 1))
                        raw = scr.tile([128, 512], BF16, tag="rawk", bufs=4,
                                       name=f"rwA{dt}_{sc}")
                        nc.scalar.activation(raw[:], pk[:], AF.Identity,
                                             bias=bcol("bk", dt))
                        if pend is not None:
                            rope_k(pend[0], sc, pend[1])
                        pend = (dt, raw)
                    # V proj for this chunk (sg = sc)
                    for sl in range(4):
                        st = sc * 4 + sl
                        va3 = VA[st].rearrange("p (h c) -> p h c", c=65)
                        nc.scalar.activation(
                            va3[:, :, 64:65],
                            ones16.rearrange("p (h c) -> p h c", c=1),
                            AF.Identity)
                        pv = psB.tile([128, 1024], F32, tag="pv", bufs=2,
                                      name=f"pv{st}")
                        for n in range(2):
                            # v bias is folded into the out_proj bias on the
                            # host (ob' = out_b + out_w @ bv): attn(v+b) =
                            # attn(v) + b exactly, since sum(probs) = 1.
                            for kt in range(NT_D):
                                nc.tensor.matmul(
                                    pv[:, n * 512:(n + 1) * 512],
                                    lhsT=xs[:, kt * 512 + sl * 128:
                                            kt * 512 + (sl + 1) * 128],
                                    rhs=wv_t[:, kt * 1024 + n * 512:
                                             kt * 1024 + (n + 1) * 512],
                                    start=(kt == 0), stop=(kt == NT_D - 1))
                            nc.scalar.activation(
                                va3[:, n * 8:(n + 1) * 8, 0:64],
                                pv[:, n * 512:(n + 1) * 512]
                                .rearrange("p (h c) -> p h c", c=64),
                                AF.Identity)
                    if pend is not None:
                        rope_k(pend[0], sc, pend[1])
            s_x.close()   # free x stream slots + wk/wv

            # ======= Phase D: attention + interleaved out_proj + LN1 stats ==
            ATT = [pATT.tile([128, Q], BF16, tag=f"att{i}", name=f"ATT{i}")
                   for i in range(NT_D)]
            HR = [hrp.tile([128, Q], F32R, tag=f"hr{i}", name=f"HR{i}")
                  for i in range(NT_D)]
            H1b = [h1p.tile([128, Q], BF16, tag=f"h1b{i}", name=f"H1b{i}")
                   for i in range(NT_D)]
            with ExitStack() as octx2:
                psStat = octx2.enter_context(
                    tc.tile_pool(name="psStatD", bufs=1, space="PSUM"))
                pSum = psStat.tile([1, Q], F32, tag="psum")
                with ExitStack() as ctx:
                    ptp = ctx.enter_context(tc.tile_pool(name="ptp", bufs=1))
                    nrm = ctx.enter_context(tc.tile_pool(name="nrm", bufs=1))
                    scr = ctx.enter_context(tc.tile_pool(name="scrD", bufs=1))
                    psS = ctx.enter_context(tc.tile_pool(name="psS", bufs=1,
                                                         space="PSUM"))
                    psAt = ctx.enter_context(tc.tile_pool(name="psAt", bufs=1,
                                                          space="PSUM"))
                    psE = ctx.enter_context(tc.tile_pool(name="psE", bufs=1,
                                                         space="PSUM"))
                    for h in range(H):
                        dt, po = h // 2, (h % 2) * 64
                        pa = psAt.tile([65, 512], F32, tag="pa", bufs=2,
                                       name=f"pa{h}")
                        for kcp in range(NT_S // 2):
                            ps_t = psS.tile([128, 1024], F32, tag="ps", bufs=2,
                                            name=f"ps{h}_{kcp}")
                            for half in range(2):
                                kc = kcp * 2 + half
                                nc.tensor.matmul(
                                    ps_t[:, half * 512:(half + 1) * 512],
                                    lhsT=KT[dt][po:po + 64,
                                                kc * 128:(kc + 1) * 128],
                                    rhs=QT[dt][po:po + 64, :],
                                    start=True, stop=True)
                            pt_t = ptp.tile([128, 1024], BF16, tag="pt",
                                            bufs=5, name=f"pt{h}_{kcp}")
                            nc.scalar.activation(pt_t[:], ps_t[:], AF.Exp,
                                                 scale=0.125)
                            for half in range(2):
                                kc = kcp * 2 + half
                                nc.tensor.matmul(
                                    pa[:],
                                    lhsT=VA[kc][:, h * 65:h * 65 + 65],
                                    rhs=pt_t[:, half * 512:(half + 1) * 512],
                                    start=(kc == 0), stop=(kc == NT_S - 1))
                        rec2 = nrm.tile([1, 512], F32, tag="rec2", bufs=2,
                                        name=f"rec2_{h}")
                        nc.vector.reciprocal(rec2[:], pa[64:65, :])
                        recb = nrm.tile([128, 512], F32, tag="recb", bufs=3,
                                        name=f"recb{h}")
                        nc.gpsimd.partition_broadcast(recb[:], rec2[:],
                                                      channels=128)
                        # psum in0 permits crossed partition bases on DVE
                        nc.vector.tensor_mul(ATT[dt][po:po + 64, :],
                                             pa[0:64, :],
                                             recb[po:po + 64, :])
                    # out_proj: po_t[ot] accumulates over at in head order, so
                    # Tile starts these matmuls as ATT tiles complete, filling
                    # PE idle while ACT grinds the exps. LN1 sum/sq stats
                    # accumulate as each HR tile lands.
                    for ot in range(NT_D):
                        po_t = psE.tile([128, Q], F32, tag="po", bufs=1,
                                        name=f"poE{ot}")
                        for at_ in range(NT_D):
                            nc.tensor.matmul(
                                po_t[:],
                                lhsT=ow_t[:, at_ * 1024 + ot * 128:
                                          at_ * 1024 + (ot + 1) * 128],
                                rhs=ATT[at_][:],
                                start=(at_ == 0), stop=(at_ == NT_D - 1))
                        # HR = (po + ob) + x  (residual; x in bf16)
                        nc.vector.scalar_tensor_tensor(
                            HR[ot][:], po_t[:], bcol("ob", ot),
                            x0_t[:, ot * 512:(ot + 1) * 512],
                            ALU.add, ALU.add)
                    # sum stats AFTER the loop: interleaved per-ot they
                    # would make the in-order PE wait on each ot's chain.
                    for ot in range(NT_D):
                        nc.tensor.matmul(pSum[:], lhsT=ones_c, rhs=HR[ot][:],
                                         start=(ot == 0), stop=(ot == NT_D - 1))
                # ---- LN1 stats + normalize (rstd via Ln+Exp) ----
                # LN1 scratch lives on the RIGHT stack (in the freed x/wkv
                # region): on the left it would occupy the slot phase F's
                # w1 pool reuses, and the w1b prefetch would WAR-wait on
                # the whole LN1 tail.
                with ExitStack() as ctx:
                    scr = ctx.enter_context(tc.tile_pool(name="scrE2", bufs=1,
                                                         side="right"))
                    stat = ctx.enter_context(tc.tile_pool(name="statE2",
                                                          bufs=1,
                                                          side="right"))
                    psSq = ctx.enter_context(tc.tile_pool(name="psSqD", bufs=1,
                                                          space="PSUM"))
                    pSq = psSq.tile([1, Q], F32, tag="psq")
                    for ot in range(NT_D):
                        sq = scr.tile([128, Q], F32R, tag="sq", bufs=2,
                                      name=f"sqE{ot}")
                        nc.vector.tensor_mul(sq[:], HR[ot][:].bitcast(F32),
                                             HR[ot][:].bitcast(F32))
                        nc.tensor.matmul(pSq[:], lhsT=ones_c, rhs=sq[:],
                                         start=(ot == 0), stop=(ot == NT_D - 1))
                    mu = stat.tile([1, Q], F32, tag="mu")
                    nc.vector.tensor_scalar_mul(mu[:], pSum[:], 1.0 / D)
                    mu2 = stat.tile([1, Q], F32, tag="mu2")
                    nc.vector.tensor_mul(mu2[:], mu[:], mu[:])
                    var = stat.tile([1, Q], F32, tag="var")
                    # var = pSq/D - mu^2 in one op; +eps folds into Ln's bias
                    nc.vector.scalar_tensor_tensor(
                        var[:], pSq[:], 1.0 / D, mu2[:],
                        ALU.mult, ALU.subtract)
                    lnv = stat.tile([1, Q], F32, tag="lnv")
                    nc.scalar.activation(lnv[:], var[:], AF.Ln,
                                         bias=eps_c)
                    rstd = stat.tile([1, Q], F32, tag="rstd")
                    nc.scalar.activation(rstd[:], lnv[:], AF.Exp, scale=-0.5)
                    muf = stat.tile([128, Q], F32, tag="muf")
                    nc.gpsimd.partition_broadcast(muf[:], mu[:], channels=128)
                    rstdf = stat.tile([128, Q], F32, tag="rstdf")
                    nc.gpsimd.partition_broadcast(rstdf[:], rstd[:],
                                                  channels=128)
                    # H1b = ((HR - mu) * g1) * rstd, WITHOUT + be1: the LN1
                    # bias is folded into b1 (b1 + w1 @ be1) and the FFN
                    # residual bias (b2 + be1) on the host. Subs alternate
                    # DVE / Pool to shorten the serial chain into FFN1.
                    for ot in range(NT_D):
                        t1 = scr.tile([128, Q], F32, tag="t1", bufs=2,
                                      name=f"t1E{ot}")
                        eng = nc.vector if ot % 2 == 0 else nc.gpsimd
                        eng.tensor_sub(t1[:], HR[ot][:].bitcast(F32), muf[:])
                        nc.vector.scalar_tensor_tensor(
                            H1b[ot][:], t1[:], bcol("g1", ot), rstdf[:],
                            ALU.mult, ALU.mult)
            s_HR.close()    # free HR
            s_big.close()   # free KT/VA/ATT/x0 before FFN tiles

            # ============ Phase F: FFN + residual + LN2 ============
            # Left-pool open order is free-time order: w1p/w2p/ffp free at
            # FFN end, scrF (LN2 yt, read by the stores) frees last. The
            # NEXT rep's deep right-stack pools (wkv/xr) physically overlap
            # the leftmost of these, so put the earliest-freeing there or
            # its prefetch DMAs serialize on this rep's store tail.
            with ExitStack() as ctx:
                # w2p first: it lands on HR's just-freed region (its DMAs WAR
                # on the LN1 subs, harmless for FFN2); w1p next lands on space
                # free since the prior rep, so w1b0 prefetches early.
                w2p = ctx.enter_context(tc.tile_pool(name="w2p", bufs=1))
                w1p = ctx.enter_context(tc.tile_pool(name="w1p", bufs=1))
                ffp = ctx.enter_context(tc.tile_pool(name="pFF", bufs=1))
                grp = ctx.enter_context(tc.tile_pool(name="grp", bufs=1))
                stat = ctx.enter_context(tc.tile_pool(name="statF", bufs=1))
                scr = ctx.enter_context(tc.tile_pool(name="scrF", bufs=1))
                psStat = ctx.enter_context(tc.tile_pool(name="psStatF", bufs=1,
                                                        space="PSUM"))
                pstat2 = psStat.tile([1, 1024], F32, tag="pstat2")
                pSum2, pSq2 = pstat2[:, 0:512], pstat2[:, 512:1024]
                FFT = [ffp.tile([128, Q], BF16, tag=f"ff{i}", name=f"FFT{i}")
                       for i in range(NT_F)]
                w1v = w1P.ap().rearrange("p (kt c) -> p kt c", c=F)
                with tc.tile_pool(name="psF", bufs=1, space="PSUM") as psF:
                    for fb in range(F // 512):
                        w1b = w1p.tile([128, NT_D * 512], BF16, tag="w1",
                                       bufs=2, name=f"w1b{fb}")
                        nc.sync.dma_start(
                            w1b[:].rearrange("p (kt c) -> p kt c", c=512),
                            w1v[:, :, fb * 512:(fb + 1) * 512])
                        for j in range(4):
                            ft = fb * 4 + j
                            pf = psF.tile([128, Q], F32, tag="pf", bufs=2,
                                          name=f"pf{ft}")
                            for kt in range(NT_D):
                                nc.tensor.matmul(
                                    pf[:],
                                    lhsT=w1b[:, kt * 512 + j * 128:
                                             kt * 512 + (j + 1) * 128],
                                    rhs=H1b[kt][:],
                                    start=(kt == 0), stop=(kt == NT_D - 1))
                            nc.scalar.activation(FFT[ft][:], pf[:], AF.Relu,
                                                 bias=bcol("b1", ft))
                # x0 reload for the next rep: emitted HERE (not right after
                # its last reader in phase D) because the trigger occupies
                # the ACT sequencer while its WAR pends — between D and LN1
                # it would stall the Ln/Exp chain.
                if _rep + 1 < repeat:
                    x0_next = load_x0()
                GR = [grp.tile([128, Q], F32R, tag=f"gr{i}", name=f"GR{i}")
                      for i in range(NT_D)]
                w2v = w2P.ap().rearrange("p (ft c) -> p ft c", c=D)
                with tc.tile_pool(name="psG", bufs=1, space="PSUM") as psG:
                    for ot in range(NT_D):
                        w2b = w2p.tile([128, NT_F * 128], BF16, tag="w2",
                                       bufs=2, name=f"w2b{ot}")
                        # w2 on the ACT ring: on SP it queues behind w1's
                        # slot-WAR-blocked preps (head-of-line) and stalls
                        # the w1->w2 transition
                        nc.scalar.dma_start(
                            w2b[:].rearrange("p (ft c) -> p ft c", c=128),
                            w2v[:, :, ot * 128:(ot + 1) * 128])
                        pg = psG.tile([128, Q], F32, tag="pg", bufs=2,
                                      name=f"pg{ot}")
                        for ft in range(NT_F):
                            nc.tensor.matmul(
                                pg[:], lhsT=w2b[:, ft * 128:(ft + 1) * 128],
                                rhs=FFT[ft][:],
                                start=(ft == 0), stop=(ft == NT_F - 1))
                        # GR = (pg + b2) + h  (residual; h in bf16)
                        nc.vector.scalar_tensor_tensor(
                            GR[ot][:], pg[:], bcol("b2", ot),
                            H1b[ot][:], ALU.add, ALU.add)
                        sq2 = scr.tile([128, Q], F32R, tag="sq2", bufs=2,
                                       name=f"sq2F{ot}")
                        nc.gpsimd.tensor_mul(sq2[:],
                                             GR[ot][:].bitcast(F32),
                                             GR[ot][:].bitcast(F32))
                        nc.tensor.matmul(pSum2, lhsT=ones_c, rhs=GR[ot][:],
                                         start=(ot == 0), stop=(ot == NT_D - 1))
                        nc.tensor.matmul(pSq2, lhsT=ones_c, rhs=sq2[:],
                                         start=(ot == 0), stop=(ot == NT_D - 1))
                mu = stat.tile([1, Q], F32, tag="mu")
                nc.vector.tensor_scalar_mul(mu[:], pSum2, 1.0 / D)
                mu2 = stat.tile([1, Q], F32, tag="mu2")
                nc.vector.tensor_mul(mu2[:], mu[:], mu[:])
                var = stat.tile([1, Q], F32, tag="var")
                nc.vector.scalar_tensor_tensor(
                    var[:], pSq2, 1.0 / D, mu2[:],
                    ALU.mult, ALU.subtract)
                lnv = stat.tile([1, Q], F32, tag="lnv")
                nc.scalar.activation(lnv[:], var[:], AF.Ln, bias=eps_c)
                rstd = stat.tile([1, Q], F32, tag="rstd")
                nc.scalar.activation(rstd[:], lnv[:], AF.Exp, scale=-0.5)
                muf = stat.tile([128, Q], F32, tag="muf")
                nc.gpsimd.partition_broadcast(muf[:], mu[:], channels=128)
                rstdf = stat.tile([128, Q], F32, tag="rstdf")
                nc.gpsimd.partition_broadcast(rstdf[:], rstd[:], channels=128)
                for ot in range(NT_D):
                    t1 = scr.tile([128, Q], F32, tag="t1f", bufs=2,
                                  name=f"t1F{ot}")
                    nc.vector.tensor_sub(t1[:], GR[ot][:].bitcast(F32), muf[:])
                    t2 = scr.tile([128, Q], F32, tag="t2f", bufs=2,
                                  name=f"t2F{ot}")
                    nc.vector.scalar_tensor_tensor(
                        t2[:], t1[:], bcol("g2", ot), rstdf[:],
                        ALU.mult, ALU.mult)
                    yt = scr.tile([128, Q], F32, tag="yt", bufs=2,
                                  name=f"ytF{ot}")
                    nc.scalar.activation(yt[:], t2[:], AF.Identity,
                                         bias=bcol("be2", ot))
                    # stores ride the SWDGE queue (Pool idle here): on either
                    # HWDGE ring their preps (gated by the serial LN2 chain)
                    # would head-of-line block the NEXT rep's x/weight loads
                    nc.gpsimd.dma_start(yT.ap()[ot * 128:(ot + 1) * 128, :],
                                        yt[:])
            s_EF.close()
            if _rep + 1 < repeat:
                x0_t, wq_t = x0_next, wq_next

    nc.compile()
    return nc


def _rope_tables():
    inv_freq = (1.0 / (ROPE_BASE ** (np.arange(0, Dh, 2, dtype=np.float32) / Dh)))
    angles = np.arange(S, dtype=np.float32)[:, None] * inv_freq[None, :]
    cos = np.cos(angles).T.astype(np.float32)   # (32, S)
    sin = np.sin(angles).T.astype(np.float32)
    cosK = np.concatenate([cos, cos, cos, cos], axis=0)          # (128, S)
    sinKA = np.concatenate([sin, sin, sin, sin], axis=0)
    return np.ascontiguousarray(cosK), np.ascontiguousarray(sinKA)


def _pack_w(wT):
    """(D, D) W^T -> (128, 8192) bf16: [p, kt*1024 + c] = wT[kt*128+p, c]."""
    w = np.asarray(wT, dtype=np.float32).reshape(NT_D, 128, D)
    return np.ascontiguousarray(
        w.transpose(1, 0, 2).reshape(128, NT_D * D).astype(ml_dtypes.bfloat16))


def _psign():
    """Signed 32-row swap as a stationary matmul operand.

    psum_sw = psignT.T @ raw gives psum_sw[m] = s_m * raw[m ^ 32], with
    s_m = -1 for (m % 64) < 32 else +1 (the -sin/+sin rope sign pattern).
    """
    p = np.zeros((128, 128), np.float32)
    for m in range(128):
        s = -1.0 if (m % 64) < 32 else 1.0
        p[m ^ 32, m] = s
    return np.ascontiguousarray(p.astype(ml_dtypes.bfloat16))


def _in_maps(x, in_proj_w, in_proj_b, out_w, out_b, w1, b1, w2, b2,
             ln1_g, ln1_b, ln2_g, ln2_b):
    x = np.asarray(x, dtype=np.float32)

    perm = np.concatenate(
        [h * Dh + np.concatenate([np.arange(0, Dh, 2), np.arange(1, Dh, 2)])
         for h in range(H)])
    wq = np.asarray(in_proj_w)[0:D][perm]
    wk = np.asarray(in_proj_w)[D:2 * D][perm]
    wv = np.asarray(in_proj_w)[2 * D:3 * D]
    bqv = np.asarray(in_proj_b)[0:D][perm]
    bkv = np.asarray(in_proj_b)[D:2 * D][perm]
    bvv = np.asarray(in_proj_b)[2 * D:3 * D]
    cosK, sinKA = _rope_tables()

    w1T = np.asarray(w1, dtype=np.float32).T          # (D, F)
    w2T = np.asarray(w2, dtype=np.float32).T          # (F, D)
    w1Pm = np.ascontiguousarray(
        w1T.reshape(NT_D, 128, F).transpose(1, 0, 2).reshape(128, NT_D * F)
        .astype(ml_dtypes.bfloat16))
    w2Pm = np.ascontiguousarray(
        w2T.reshape(NT_F, 128, D).transpose(1, 0, 2).reshape(128, NT_F * D)
        .astype(ml_dtypes.bfloat16))

    bpack = np.zeros((128, BP_COLS), np.float32)

    def put(key, vec):
        v = np.asarray(vec, dtype=np.float32).reshape(-1)
        n = v.size // 128
        bpack[:, _BP[key]:_BP[key] + n] = v.reshape(n, 128).T
    ob_eff = (np.asarray(out_b, dtype=np.float64) +
              np.asarray(out_w, dtype=np.float64) @
              np.asarray(bvv, dtype=np.float64)).astype(np.float32)
    # LN1's +be1 is folded out of the kernel: h = t2 + be1 with t2 the
    # kernel's H1b, so ffn1 uses b1 + w1 @ be1 and the ffn residual uses
    # b2 + be1 (exact).
    be1v = np.asarray(ln1_b, dtype=np.float64)
    b1_eff = (np.asarray(b1, dtype=np.float64) +
              np.asarray(w1, dtype=np.float64) @ be1v).astype(np.float32)
    b2_eff = (np.asarray(b2, dtype=np.float64) + be1v).astype(np.float32)
    put("bq", bqv); put("bk", bkv); put("ob", ob_eff); put("b2", b2_eff)
    put("g1", ln1_g); put("be1", ln1_b); put("g2", ln2_g); put("be2", ln2_b)
    put("b1", b1_eff)
    bpack[:, _BP["eps"]] = LN_EPS
    bpack[:, _BP["ones"]:_BP["ones"] + 16] = 1.0

    shared = {
        "wqP": _pack_w(wq.T), "wkP": _pack_w(wk.T), "wvP": _pack_w(wv.T),
        "owP": _pack_w(np.asarray(out_w).T),
        "w1P": w1Pm, "w2P": w2Pm,
        "bpack": bpack, "psignT": _psign(),
        "identT": np.ascontiguousarray(
            np.eye(128, dtype=np.float32).astype(ml_dtypes.bfloat16)),
    }
    in_maps = []
    for c in range(8):
        b_, qb = c // 4, c % 4
        xT = x[b_].T                                   # (D, S)
        # packed x: chunk order [qb, others...]; chunk i block kt at
        # cols i*4096 + kt*512; K rope tables permuted to match
        order = [qb] + [s for s in range(NC_S) if s != qb]
        xPm = np.empty((128, NC_S * 4096), np.float32)
        cosKP = np.empty((128, S), np.float32)
        sinKP = np.empty((128, S), np.float32)
        for i, sc in enumerate(order):
            blk = xT[:, sc * 512:(sc + 1) * 512]       # (D, 512)
            xPm[:, i * 4096:(i + 1) * 4096] = (
                blk.reshape(NT_D, 128, 512).transpose(1, 0, 2)
                .reshape(128, 4096))
            cosKP[:, i * 512:(i + 1) * 512] = cosK[:, sc * 512:(sc + 1) * 512]
            sinKP[:, i * 512:(i + 1) * 512] = sinKA[:, sc * 512:(sc + 1) * 512]
        m = dict(shared)
        m["xPb"] = np.ascontiguousarray(xPm.astype(ml_dtypes.bfloat16))
        m["cosKb"] = np.ascontiguousarray(cosKP.astype(ml_dtypes.bfloat16))
        m["sinKAb"] = np.ascontiguousarray(sinKP.astype(ml_dtypes.bfloat16))
        in_maps.append(m)
    return in_maps


def kernel(x, in_proj_w, in_proj_b, out_w, out_b, w1, b1, w2, b2,
           ln1_g, ln1_b, ln2_g, ln2_b):
    if "nc" not in _CACHE:
        _CACHE["nc"] = _build()
    nc = _CACHE["nc"]
    in_maps = _in_maps(x, in_proj_w, in_proj_b, out_w, out_b, w1, b1, w2, b2,
                       ln1_g, ln1_b, ln2_g, ln2_b)
    res = run_bass_kernel_spmd(nc, in_maps, core_ids=list(range(8)))
    out = np.empty((B, S, D), dtype=np.float32)
    for c in range(8):
        b_, qb = c // 4, c % 4
        out[b_, qb * Q:(qb + 1) * Q, :] = res.results[c]["yT"].T
    return out
